# revision 16
# baseline (speedup 1.0000x reference)
"""Trainium2 Bass kernel v14 for nn_CrossMambaFusionBlock.

8 cores = 4 samples x 2 modalities. Changes vs v13:
  - phase restructure: P1 = stage A + k0 + k2 interleaved (fills the DVE
    idle that stage A left), P2 = k1 + k3, P3 = stage C chunks.
    stage A emits l-chunks in order 0,3,2,1 so k0 (fwd) and k2 (rev)
    can both start after two chunks.
  - stage A: conv bias folded into the Silu activation bias -> no init
    matmuls (-32 matmuls).
  - pair-tile (d1) scans moved to GpSimd (it also implements
    tensor_tensor_scan); d0 scans stay on Vector.
  - ydst1 first-write copies moved from GpSimd (6.7us each!) to Act.
  - per-direction-slot pool tags with bufs=1 (same SBUF as v13's
    bufs=2 shared tags) so two directions pipeline concurrently.
  - all directions use the PSUM identity-matmul readout path
    (PSUM p1: stage A 2 + prep 5 + yps 1 = 8 banks exactly).
  - prep split into prep_proj (PE/scalar/DMA) and prep_dtu_b
    (dtu + C broadcast + b-mults) to keep engine FIFOs unblocked.
"""

import sys
import types
from contextlib import ExitStack

import ml_dtypes
import numpy as np

BF = ml_dtypes.bfloat16

B, H, W, C = 4, 64, 64, 96
DIN = 192
N = 4
R = 6
K = 4
L = H * W
D0, D1 = 128, 64
NCORE = 8
LCH = 1024
NCH = L // LCH
MMCH = 512
LN_EPS = 1e-5
PADW = (H + 2) * (W + 2)


def _install_ntff_hook():
    if "antenv.axon_hooks" in sys.modules:
        return
    try:
        import antenv.axon_hooks  # noqa: F401
        return
    except ImportError:
        pass
    try:
        mod = types.ModuleType("antenv.axon_hooks")
        _h = [None]
        mod.set_axon_ntff_profile_hook = lambda h: _h.__setitem__(0, h)
        mod.get_axon_ntff_profile_hook = lambda: _h[0]
        sys.modules["antenv.axon_hooks"] = mod
        import antenv

        antenv.axon_hooks = mod
        from trn_agent_boot.trn_boot import _ntff_profile_via_ctypes

        mod.set_axon_ntff_profile_hook(
            _ntff_profile_via_ctypes("/opt/axon/libaxon_pjrt.so")
        )
    except Exception:
        pass


_install_ntff_hook()

import concourse.hw_specs as _hw_specs  # noqa: E402

_orig_get_act_tables = _hw_specs.get_activation_tables


def _steered_act_tables(module_arch):
    tabs = _orig_get_act_tables(module_arch)
    import concourse.mybir as _mb

    combined = "natural_log_exp_and_others"
    if combined in tabs:
        for name, fns in tabs.items():
            if name != combined:
                fns.discard(_mb.ActivationFunctionType.Exp)
                fns.discard(_mb.ActivationFunctionType.Ln)
    return tabs


_hw_specs.get_activation_tables = _steered_act_tables

import concourse.bacc as bacc  # noqa: E402
import concourse.bass as bass  # noqa: E402
import concourse.mybir as mybir  # noqa: E402
import concourse.tile as tile  # noqa: E402
from concourse.bass_utils import run_bass_kernel_spmd  # noqa: E402

F32 = mybir.dt.float32
BF16 = mybir.dt.bfloat16
MUL = mybir.AluOpType.mult
ADD = mybir.AluOpType.add
SUB = mybir.AluOpType.subtract
AF = mybir.ActivationFunctionType

CFG = {"use_silu": True}


def _bc0(row_ap, nrep):
    """[1, F] row -> [nrep, F] DMA replication source AP."""
    return bass.AP(tensor=row_ap.tensor, offset=row_ap.offset,
                   ap=[[0, nrep]] + list(row_ap.ap)[1:])


def _rep_mid(ap2d, nrep):
    """[p, F] tile AP -> [p, nrep, F] view with stride-0 middle dim."""
    a = list(ap2d.ap)
    return bass.AP(tensor=ap2d.tensor, offset=ap2d.offset,
                   ap=[a[0], [0, nrep]] + a[1:])


def _v3(ap2d):
    return ap2d.rearrange("p (a b) -> p a b", b=H)


def build_nc():
    nc = bacc.Bacc("TRN2", target_bir_lowering=False, debug=False,
                   num_devices=NCORE)

    def din(name, shape, dt=BF16):
        return nc.dram_tensor(name, shape, dt, kind="ExternalInput").ap()

    xpad_o = din("xpad_o", [C, PADW])
    xpad_t = din("xpad_t", [C, PADW])
    xnat_o = din("xnat_o", [L, C], F32)
    wf_o = din("wf_o", [C, 9 * 256])
    wf_t = din("wf_t", [C, 9 * 256])
    cb_o = din("cb_o", [1, 256])
    cb_t = din("cb_t", [1, 256])
    cbc_o = din("cbc_o", [128, 2], F32)
    cbc_t = din("cbc_t", [128, 2], F32)
    xpw_dt0 = din("xpw_dt0", [D0, K * R])
    xpw_dt1 = din("xpw_dt1", [D1, K * R])
    xpw_B0 = din("xpw_B0", [D0, K * 128])
    xpw_B1 = din("xpw_B1", [D1, K * 128])
    xpw_C0 = din("xpw_C0", [D0, K * 128])
    xpw_C1 = din("xpw_C1", [D1, K * 128])
    dtw_d0 = din("dtw_d0", [R, K * D0])
    dtw_d1p = din("dtw_d1p", [R, K * 128])
    dtb_d0 = din("dtb_d0", [D0, K], F32)
    dtb_d1p = din("dtb_d1p", [128, K], F32)
    nscale_d1a = din("nscale_d1a", [128, 1], F32)
    nscale_d1b = din("nscale_d1b", [128, 1], F32)
    ident128 = din("ident128", [D0, 128])
    shsum = din("shsum", [128, 64])
    dsum_d0 = din("dsum_d0", [D0, 1], F32)
    dsum_d1 = din("dsum_d1", [D1, 1], F32)
    ln_g0 = din("ln_g0", [D0, 1], F32)
    ln_g1 = din("ln_g1", [D1, 1], F32)
    ln_b0 = din("ln_b0", [D0, 1], F32)
    ln_b1 = din("ln_b1", [D1, 1], F32)
    woutT0 = din("woutT0", [D0, C])
    woutT1 = din("woutT1", [D1, C])
    out_o = nc.dram_tensor("out_o", [L, C], F32, kind="ExternalOutput").ap()
    bc_stage = nc.dram_tensor("bc_stage", [K, NCH, 8, LCH], BF16,
                              kind="Internal").ap()

    with tile.TileContext(nc, num_cores=NCORE, pool_alloc_mode="queue") as tc, \
            ExitStack() as ctx:
        cpool = ctx.enter_context(tc.tile_pool(name="consts", bufs=1))

        _qrot = [nc.sync, nc.scalar]
        _qi = [0]

        def ctile(name, src, shape, dt=BF16):
            t = cpool.tile(shape, dt, name=name)
            _qrot[_qi[0] % 2].dma_start(t[:], src)
            _qi[0] += 1
            return t

        w_xpw_dt0 = ctile("w_xpw_dt0", xpw_dt0[:], [D0, K * R])
        w_xpw_dt1 = ctile("w_xpw_dt1", xpw_dt1[:], [D1, K * R])
        w_xpw_B0 = ctile("w_xpw_B0", xpw_B0[:], [D0, K * 128])
        w_xpw_B1 = ctile("w_xpw_B1", xpw_B1[:], [D1, K * 128])
        w_xpw_C0 = ctile("w_xpw_C0", xpw_C0[:], [D0, K * 128])
        w_xpw_C1 = ctile("w_xpw_C1", xpw_C1[:], [D1, K * 128])
        w_dtw_d0 = ctile("w_dtw_d0", dtw_d0[:], [R, K * D0])
        w_dtw_d1p = ctile("w_dtw_d1p", dtw_d1p[:], [R, K * 128])
        w_dtb_d0 = ctile("w_dtb_d0", dtb_d0[:], [D0, K], F32)
        w_dtb_d1p = ctile("w_dtb_d1p", dtb_d1p[:], [128, K], F32)
        w_nsa = ctile("w_nsa", nscale_d1a[:], [128, 1], F32)
        w_nsb = ctile("w_nsb", nscale_d1b[:], [128, 1], F32)
        w_ident = ctile("w_ident", ident128[:], [D0, 128])
        w_shsum = ctile("w_shsum", shsum[:], [128, 64])
        w_dsum0 = ctile("w_dsum0", dsum_d0[:], [D0, 1], F32)
        w_dsum1 = ctile("w_dsum1", dsum_d1[:], [D1, 1], F32)
        w_lng0 = ctile("w_lng0", ln_g0[:], [D0, 1], F32)
        w_lng1 = ctile("w_lng1", ln_g1[:], [D1, 1], F32)
        w_lnb0 = ctile("w_lnb0", ln_b0[:], [D0, 1], F32)
        w_lnb1 = ctile("w_lnb1", ln_b1[:], [D1, 1], F32)
        w_woutT0 = ctile("w_woutT0", woutT0[:], [D0, C])
        w_woutT1 = ctile("w_woutT1", woutT1[:], [D1, C])
        w_cb_o = ctile("w_cb_o", cb_o[:], [1, 256])
        w_cb_t = ctile("w_cb_t", cb_t[:], [1, 256])
        w_cbc_o = ctile("w_cbc_o", cbc_o[:], [128, 2], F32)
        w_cbc_t = ctile("w_cbc_t", cbc_t[:], [128, 2], F32)
        ones_row = cpool.tile([1, MMCH], BF16, name="ones_row")
        nc.vector.memset(ones_row[:], 1.0)
        mean_l0 = cpool.tile([D0, 128], BF16, name="mean_l0")
        nc.vector.memset(mean_l0[:], 1.0 / DIN)
        mean_l1 = cpool.tile([D1, 128], BF16, name="mean_l1")
        nc.vector.memset(mean_l1[:], 1.0 / DIN)
        eps_col = cpool.tile([128, 1], F32, name="eps_col")
        nc.vector.memset(eps_col[:], LN_EPS)

        big = ctx.enter_context(tc.tile_pool(name="big", bufs=1))
        u_o_d0 = big.tile([D0, L], BF16, name="u_o_d0")
        u_o_d1p = big.tile([128, L], BF16, name="u_o_d1p")
        u_t_d0 = big.tile([D0, L], BF16, name="u_t_d0")
        u_t_d1p = big.tile([128, L], BF16, name="u_t_d1p")
        y02_d0 = big.tile([D0, L], BF16, name="y02_d0")
        y13_d0 = big.tile([D0, L], BF16, name="y13_d0")
        y02_d1p = big.tile([128, L], BF16, name="y02_d1p")
        y13_d1p = big.tile([128, L], BF16, name="y13_d1p")
        # per-(k, n) scan carries parked here so h tiles release on read_iter
        carry_t = big.tile([128, K * 8], F32, name="carry_t")

        # stage B SBUF pools: one buffer per direction-slot tag.
        # slot 0 hosts k0 (P1) then k1 (P2); slot 1 hosts k2 then k3.
        cmp_p = ctx.enter_context(tc.tile_pool(name="cmp", bufs=2))
        dtp = ctx.enter_context(tc.tile_pool(name="dtp", bufs=2))
        bcp = ctx.enter_context(tc.tile_pool(name="bcp", bufs=2))
        abp = ctx.enter_context(tc.tile_pool(name="abp", bufs=2))
        hp = ctx.enter_context(tc.tile_pool(name="hp", bufs=2))
        bps = ctx.enter_context(tc.tile_pool(name="bps", bufs=1, space="PSUM"))
        yps = ctx.enter_context(tc.tile_pool(name="yps", bufs=1, space="PSUM"))

        carries = {}

        def u_view(u_tile, k, c, part=None):
            tl = u_tile[:part, :] if part else u_tile[:]
            if k in (0, 2):
                lc = c if k == 0 else NCH - 1 - c
                return tl[:, lc * LCH:(lc + 1) * LCH]
            wv = tl.rearrange("p (h w) -> p w h", w=W)
            wc = c if k == 1 else NCH - 1 - c
            nwc = LCH // H
            return wv[:, wc * nwc:(wc + 1) * nwc, :]

        preps = {}

        def prep_proj(k, c):
            """Projections, softplus, decay powers, bounce, B broadcasts.

            Only PE / Scalar / DMA work — no Vector or GpSimd ops, so the
            scan queues never head-of-line block on this."""
            B_sb = cmp_p.tile([128, LCH], BF16, name="B_sb", tag="B_sb")
            C_sb = cmp_p.tile([128, LCH], BF16, name="C_sb", tag="C_sb")
            sdt0 = dtp.tile([D0, LCH], BF16, name="sdt0", tag="sdt0")
            sdt1 = dtp.tile([128, LCH], BF16, name="sdt1", tag="sdt1")
            for mi in range(LCH // MMCH):
                ms = slice(mi * MMCH, (mi + 1) * MMCH)
                if k in (0, 2):
                    ro0 = u_view(u_o_d0, k, c)[:, ms]
                    ro1 = u_view(u_o_d1p, k, c, part=D1)[:, ms]
                    rt0 = u_view(u_t_d0, k, c)[:, ms]
                    rt1 = u_view(u_t_d1p, k, c, part=D1)[:, ms]
                else:
                    nw = MMCH // H
                    s3 = slice(mi * nw, (mi + 1) * nw)
                    ro0 = u_view(u_o_d0, k, c)[:, s3, :]
                    ro1 = u_view(u_o_d1p, k, c, part=D1)[:, s3, :]
                    rt0 = u_view(u_t_d0, k, c)[:, s3, :]
                    rt1 = u_view(u_t_d1p, k, c, part=D1)[:, s3, :]
                dt6 = cmp_p.tile([R, MMCH], BF16, name="dt6", tag="dt6")
                ps_dt6 = bps.tile([R, MMCH], F32, name="ps_dt6", tag="ps_dt6")
                nc.tensor.matmul(ps_dt6[:], w_xpw_dt0[:, k * R:(k + 1) * R],
                                 ro0, start=True, stop=False)
                nc.tensor.matmul(ps_dt6[:], w_xpw_dt1[:, k * R:(k + 1) * R],
                                 ro1, start=False, stop=True)
                nc.scalar.copy(dt6[:], ps_dt6[:])
                ps_B = bps.tile([128, MMCH], F32, name="ps_B", tag="ps_B")
                nc.tensor.matmul(ps_B[:], w_xpw_B0[:, k * 128:(k + 1) * 128],
                                 ro0, start=True, stop=False)
                nc.tensor.matmul(ps_B[:], w_xpw_B1[:, k * 128:(k + 1) * 128],
                                 ro1, start=False, stop=True)
                nc.scalar.copy(B_sb[:, ms], ps_B[:])
                ps_C = bps.tile([128, MMCH], F32, name="ps_C", tag="ps_C")
                nc.tensor.matmul(ps_C[:], w_xpw_C0[:, k * 128:(k + 1) * 128],
                                 rt0, start=True, stop=False)
                nc.tensor.matmul(ps_C[:], w_xpw_C1[:, k * 128:(k + 1) * 128],
                                 rt1, start=False, stop=True)
                nc.scalar.copy(C_sb[:, ms], ps_C[:])
                ps_dt0 = bps.tile([D0, MMCH], F32, name="ps_dt0",
                                  tag="ps_dt0")
                nc.tensor.matmul(ps_dt0[:], w_dtw_d0[:, k * D0:(k + 1) * D0],
                                 dt6[:], start=True, stop=True)
                nc.scalar.activation(sdt0[:, ms], ps_dt0[:], AF.Exp,
                                     bias=w_dtb_d0[:, k:k + 1], scale=1.0)
                ps_dt1 = bps.tile([128, MMCH], F32, name="ps_dt1",
                                  tag="ps_dt1")
                nc.tensor.matmul(ps_dt1[:],
                                 w_dtw_d1p[:, k * 128:(k + 1) * 128],
                                 dt6[:], start=True, stop=True)
                nc.scalar.activation(sdt1[:, ms], ps_dt1[:], AF.Exp,
                                     bias=w_dtb_d1p[:, k:k + 1], scale=1.0)
            # softplus: dt = ln(1 + e1), in place
            nc.scalar.activation(sdt0[:], sdt0[:], AF.Ln, bias=1.0)
            nc.scalar.activation(sdt1[:], sdt1[:], AF.Ln, bias=1.0)

            # ---- B/C bounce stores (B_sb/C_sb dead for SBUF after this) ----
            stg = bc_stage[k, c]
            bsrc = bass.AP(tensor=B_sb.tensor, offset=B_sb[:].offset,
                           ap=[[32 * LCH, 4]] + list(B_sb[:].ap)[1:])
            nc.sync.dma_start(stg[0:4, :], bsrc)
            csrc = bass.AP(tensor=C_sb.tensor, offset=C_sb[:].offset,
                           ap=[[32 * LCH, 4]] + list(C_sb[:].ap)[1:])
            nc.scalar.dma_start(stg[4:8, :], csrc)

            # ---- decay powers from Act ----
            a_d0 = abp.tile([D0, N * LCH], BF16, name="a_d0", tag="a_d0")
            for n in range(N):
                nc.scalar.activation(a_d0[:, n * LCH:(n + 1) * LCH], sdt0[:],
                                     AF.Exp, bias=0.0, scale=-float(n + 1))
            a_d1 = abp.tile([128, 2 * LCH], BF16, name="a_d1", tag="a_d1")
            nc.scalar.activation(a_d1[:, 0:LCH], sdt1[:], AF.Exp,
                                 bias=0.0, scale=w_nsa[:])
            nc.scalar.activation(a_d1[:, LCH:2 * LCH], sdt1[:], AF.Exp,
                                 bias=0.0, scale=w_nsb[:])

            # ---- B broadcast loads ----
            qeng = [nc.sync, nc.scalar, nc.sync, nc.scalar]
            B_all = bcp.tile([D0, N * LCH], BF16, name="B_all",
                             tag="B_all")
            C_all = bcp.tile([D0, N * LCH], BF16, name="C_all",
                             tag="C_all")
            for n in range(N):
                qeng[n % 4].dma_start(B_all[:, n * LCH:(n + 1) * LCH],
                                      _bc0(stg[n:n + 1, :], D0))
            B_p = bcp.tile([128, 2 * LCH], BF16, name="B_p", tag="B_p")
            C_p = bcp.tile([128, 2 * LCH], BF16, name="C_p", tag="C_p")
            for j in range(2):
                for half in range(2):
                    n = 2 * j + half
                    hs = slice(64 * half, 64 * half + 64)
                    jl = slice(j * LCH, (j + 1) * LCH)
                    qeng[(2 + j + half) % 4].dma_start(
                        B_p[hs, jl], _bc0(stg[n:n + 1, :], 64))

            preps[(k, c)] = [a_d0, a_d1, B_all, C_all, B_p, C_p,
                             B_sb, C_sb, sdt0, sdt1]

        def _load_c(k, c, C_all, C_p):
            stg = bc_stage[k, c]
            qeng = [nc.sync, nc.scalar, nc.sync, nc.scalar]
            for n in range(N):
                qeng[(n + 1) % 4].dma_start(C_all[:, n * LCH:(n + 1) * LCH],
                                            _bc0(stg[4 + n:5 + n, :], D0))
            for j in range(2):
                for half in range(2):
                    n = 2 * j + half
                    hs = slice(64 * half, 64 * half + 64)
                    jl = slice(j * LCH, (j + 1) * LCH)
                    qeng[(3 + j + half) % 4].dma_start(
                        C_p[hs, jl], _bc0(stg[4 + n:5 + n, :], 64))

        def prep_dtu_b(k, c):
            """dtu into the dead B_sb/C_sb tiles, C broadcasts, b = dtu*B."""
            (a_d0, a_d1, B_all, C_all, B_p, C_p,
             B_sb, C_sb, sdt0, sdt1) = preps[(k, c)]
            uvo0 = u_view(u_o_d0, k, c)
            uvo1 = u_view(u_o_d1p, k, c)
            dtu0 = B_sb
            dtu1 = C_sb
            if k in (0, 2):
                nc.gpsimd.tensor_tensor(dtu0[:], sdt0[:], uvo0, MUL)
                nc.gpsimd.tensor_tensor(dtu1[:], sdt1[:], uvo1, MUL)
            else:
                nc.vector.tensor_tensor(_v3(dtu0[:]), _v3(sdt0[:]), uvo0, MUL)
                nc.vector.tensor_tensor(_v3(dtu1[:]), _v3(sdt1[:]), uvo1, MUL)
            _load_c(k, c, C_all, C_p)
            nc.vector.tensor_tensor(
                B_all[:].rearrange("p (n l) -> p n l", n=N),
                _rep_mid(dtu0[:], N),
                B_all[:].rearrange("p (n l) -> p n l", n=N), MUL)
            nc.vector.tensor_tensor(
                B_p[:].rearrange("p (n l) -> p n l", n=2),
                _rep_mid(dtu1[:], 2),
                B_p[:].rearrange("p (n l) -> p n l", n=2), MUL)
            preps[(k, c)] = [a_d0, a_d1, B_all, C_all, B_p, C_p]

        def scans_iter(k, c):
            rev = k >= 2
            a_d0, a_d1, B_all, C_all, B_p, C_p = preps[(k, c)]
            h_d0 = hp.tile([D0, N * LCH], BF16, name="h_d0", tag="h_d0")
            h_p = hp.tile([128, 2 * LCH], BF16, name="h_p", tag="h_p")
            for grp, htile, atile, btile, eng, joff in (
                (range(N), h_d0, a_d0, B_all, nc.vector, 0),
                (range(2), h_p, a_d1, B_p, nc.vector, 4),
            ):
                ng = len(grp)
                for n in grp:
                    key = (k, joff, n)
                    init = carries.get(key, 0.0)
                    sl = slice(n * LCH, (n + 1) * LCH)
                    if not rev:
                        eng.tensor_tensor_scan(
                            htile[:, sl], atile[:, sl], btile[:, sl],
                            init, MUL, ADD)
                    else:
                        eng.tensor_tensor_scan(
                            htile[:, sl][:, ::-1], atile[:, sl][:, ::-1],
                            btile[:, sl][:, ::-1], init, MUL, ADD)
                    carries[key] = carry_t[:, k * 8 + joff + n:
                                           k * 8 + joff + n + 1]
                # park the group's carries in carry_t with one strided copy
                hv = htile[:].rearrange("p (n l) -> p n l", l=LCH)
                csrc3 = hv[:, :, 0:1] if rev else hv[:, :, LCH - 1:LCH]
                nc.vector.tensor_copy(
                    carry_t[:, k * 8 + joff:k * 8 + joff + ng],
                    csrc3.rearrange("p n o -> p (n o)"))
            preps[(k, c)] += [h_d0, h_p]

        def read_iter(k, c, first, ybufs, yps=None):
            lc = c if k in (0, 1) else NCH - 1 - c
            csl = slice(lc * LCH, (lc + 1) * LCH)
            ydst0 = y02_d0 if k in (0, 2) else y13_d0
            ydst1 = y02_d1p if k in (0, 2) else y13_d1p
            a_d0, a_d1, B_all, C_all, B_p, C_p, h_d0, h_p = preps.pop((k, c))

            # hc in place over C
            nc.vector.tensor_tensor(C_all[:], h_d0[:], C_all[:], MUL)
            nc.vector.tensor_tensor(C_p[:], h_p[:], C_p[:], MUL)
            nc.vector.tensor_tensor(C_p[:, 0:LCH], C_p[:, 0:LCH],
                                    C_p[:, LCH:2 * LCH], ADD)
            if first:
                nc.scalar.copy(ydst1[:, csl], C_p[:, 0:LCH])
            else:
                nc.gpsimd.tensor_tensor(ydst1[:, csl], ydst1[:, csl],
                                        C_p[:, 0:LCH], ADD)

            if ybufs == 0:
                nc.vector.tensor_tensor(
                    C_all[:, 0:2 * LCH].rearrange("p (a l) -> p a l", a=2),
                    C_all[:, 0:2 * LCH].rearrange("p (a l) -> p a l", a=2),
                    C_all[:, 2 * LCH:4 * LCH].rearrange("p (a l) -> p a l",
                                                        a=2), ADD)
                if first:
                    nc.vector.tensor_tensor(ydst0[:, csl], C_all[:, 0:LCH],
                                            C_all[:, LCH:2 * LCH], ADD)
                else:
                    nc.vector.tensor_tensor(C_all[:, 0:LCH], C_all[:, 0:LCH],
                                            C_all[:, LCH:2 * LCH], ADD)
                    nc.vector.tensor_tensor(ydst0[:, csl], ydst0[:, csl],
                                            C_all[:, 0:LCH], ADD)
            else:
                for hh in range(2):
                    ysl = slice(lc * LCH + hh * MMCH,
                                lc * LCH + (hh + 1) * MMCH)
                    ps_y = yps.tile([D0, MMCH], F32, name="ps_y", tag="ps_y0")
                    for n in range(N):
                        nc.tensor.matmul(
                            ps_y[:], w_ident[:],
                            C_all[:, n * LCH + hh * MMCH:
                                  n * LCH + hh * MMCH + MMCH],
                            start=(n == 0), stop=(n == N - 1))
                    if first:
                        nc.scalar.copy(ydst0[:, ysl], ps_y[:])
                    else:
                        nc.vector.tensor_tensor(ydst0[:, ysl], ydst0[:, ysl],
                                                ps_y[:], ADD)

        # ================= P1: stage A + k0 + k2 ===========================
        with tc.tile_pool(name="stAw", bufs=1) as awpool, \
             tc.tile_pool(name="stAx", bufs=2) as axpool, \
             tc.tile_pool(name="stAps", bufs=1, space="PSUM") as apsum:
            w_wf_o = awpool.tile([C, 9 * 256], BF16, name="w_wf_o")
            w_wf_t = awpool.tile([C, 9 * 256], BF16, name="w_wf_t")
            for si in range(4):
                ssl = slice(si * 576, (si + 1) * 576)
                _qrot[si % 2].dma_start(w_wf_o[:, ssl], wf_o[:, ssl])
                _qrot[(si + 1) % 2].dma_start(w_wf_t[:, ssl], wf_t[:, ssl])

            def stage_a_chunk(ci2):
                nrow = MMCH // W
                rows = 2 * nrow + 2
                for mod_i, (xsrc, w_wf, w_cb, w_cbc, u_d0, u_d1p) in \
                        enumerate((
                            (xpad_o, w_wf_o, w_cb_o, w_cbc_o, u_o_d0,
                             u_o_d1p),
                            (xpad_t, w_wf_t, w_cb_t, w_cbc_t, u_t_d0,
                             u_t_d1p),
                        )):
                    xp = axpool.tile([C, rows * (W + 2)], BF16,
                                     name="xp", tag=f"xp{mod_i}")
                    r0 = ci2 * 2 * nrow
                    half = (rows * (W + 2)) // 2
                    nc.sync.dma_start(
                        xp[:, 0:half],
                        xsrc[:, r0 * (W + 2):r0 * (W + 2) + half])
                    nc.scalar.dma_start(
                        xp[:, half:rows * (W + 2)],
                        xsrc[:, r0 * (W + 2) + half:(r0 + rows) * (W + 2)])
                    xv = xp[:].rearrange("c (r q) -> c r q", q=W + 2)
                    for gi, (dof, u_dst) in enumerate(((0, u_d0),
                                                      (128, u_d1p))):
                        pss = []
                        for mi in range(2):
                            ps = apsum.tile([128, MMCH], F32, name="ps_a",
                                            tag=f"ps_a{mi}")
                            if not CFG["use_silu"]:
                                nc.tensor.matmul(ps[:],
                                                 w_cb[:, dof:dof + 128],
                                                 ones_row[:], start=True,
                                                 stop=False)
                            pss.append(ps)
                        for tap in range(9):
                            dy, dx = tap // 3, tap % 3
                            wl = w_wf[:, tap * 256 + dof:tap * 256 + dof
                                      + 128]
                            for mi in range(2):
                                rhs = xv[:, mi * nrow + dy:
                                         mi * nrow + dy + nrow, dx:dx + W]
                                nc.tensor.matmul(
                                    pss[mi][:], wl, rhs,
                                    start=(CFG["use_silu"] and tap == 0),
                                    stop=(tap == 8))
                        for mi in range(2):
                            sl = slice((ci2 * 2 + mi) * MMCH,
                                       (ci2 * 2 + mi + 1) * MMCH)
                            if CFG["use_silu"]:
                                nc.scalar.activation(
                                    u_dst[:, sl], pss[mi][:], AF.Silu,
                                    bias=w_cbc[:, gi:gi + 1], scale=1.0)
                            else:
                                nc.scalar.activation(u_dst[:, sl], pss[mi][:],
                                                     AF.Sigmoid, bias=0.0,
                                                     scale=1.0)
                                nc.vector.tensor_tensor(u_dst[:, sl],
                                                        u_dst[:, sl],
                                                        pss[mi][:], MUL)

            stage_a_chunk(0)
            stage_a_chunk(3)
            seq1 = []
            for c in range(NCH):
                seq1 += [(0, c), (2, c)]
            prep_proj(*seq1[0])
            prep_dtu_b(*seq1[0])
            for i, (k, c) in enumerate(seq1):
                if i == 0:
                    stage_a_chunk(2)
                    stage_a_chunk(1)
                if i + 1 < len(seq1):
                    prep_proj(*seq1[i + 1])
                scans_iter(k, c)
                if i + 1 < len(seq1):
                    prep_dtu_b(*seq1[i + 1])
                read_iter(k, c, first=(c < 2), ybufs=1, yps=yps)

        # ================= P2: k1 + k3 =====================================
        seq2 = []
        for c in range(NCH):
            seq2 += [(1, c), (3, c)]
        prep_proj(*seq2[0])
        prep_dtu_b(*seq2[0])
        for i, (k, c) in enumerate(seq2):
            if i + 1 < len(seq2):
                prep_proj(*seq2[i + 1])
            scans_iter(k, c)
            if i + 1 < len(seq2):
                prep_dtu_b(*seq2[i + 1])
            read_iter(k, c, first=(c < 2), ybufs=1, yps=yps)

        # ================= P3: stage C =====================================
        with tc.tile_pool(name="cpl", bufs=1) as cpl, \
             tc.tile_pool(name="outp", bufs=2) as opool, \
             tc.tile_pool(name="cps", bufs=1, space="PSUM") as cps:

            def stage_c_chunk(lc):
                csl = slice(lc * LCH, (lc + 1) * LCH)
                hv0 = lc * (LCH // W)
                y13T0 = y13_d0[:].rearrange("p (w h) -> p h w", w=W)[
                    :, hv0:hv0 + LCH // W, :]
                y_bf0 = cpl.tile([D0, LCH], BF16, name="y_bf0", tag="y_bf0")
                nc.vector.tensor_tensor(
                    y_bf0[:].rearrange("p (h w) -> p h w", w=W),
                    y02_d0[:, csl].rearrange("p (h w) -> p h w", w=W),
                    y13T0, ADD)
                nc.vector.affine_then_add(y_bf0[:], u_o_d0[:, csl], y_bf0[:],
                                          w_dsum0[:], 0.0)
                y13T1 = y13_d1p[:].rearrange("p (w h) -> p h w", w=W)
                y_bf1 = cpl.tile([D1, LCH], BF16, name="y_bf1", tag="y_bf1")
                for hh in range(2):
                    ps1 = cps.tile([128, MMCH], F32, name="ps1", tag="c_p2")
                    msl = slice(lc * LCH + hh * MMCH,
                                lc * LCH + (hh + 1) * MMCH)
                    nc.tensor.matmul(ps1[:D1, :], w_shsum[:], y02_d1p[:, msl],
                                     start=True, stop=False)
                    h0 = lc * (LCH // W) + hh * (MMCH // W)
                    nc.tensor.matmul(ps1[:D1, :], w_shsum[:],
                                     y13T1[:, h0:h0 + MMCH // W, :],
                                     start=False, stop=True)
                    hs = slice(hh * MMCH, (hh + 1) * MMCH)
                    nc.vector.affine_then_add(
                        y_bf1[:, hs], u_o_d1p[0:D1, csl][:, hs], ps1[:D1, :],
                        w_dsum1[:], 0.0)
                y2_bf0 = cpl.tile([D0, LCH], BF16, name="y2_bf0",
                                  tag="y2_bf0")
                nc.scalar.activation(y2_bf0[:], y_bf0[:], AF.Square)
                y2_bf1 = cpl.tile([D1, LCH], BF16, name="y2_bf1",
                                  tag="y2_bf1")
                nc.scalar.activation(y2_bf1[:], y_bf1[:], AF.Square)

                lny0 = cpl.tile([D0, LCH], BF16, name="lny0", tag="lny0")
                lny1 = cpl.tile([D1, LCH], BF16, name="lny1", tag="lny1")
                for mi in range(2):
                    ms = slice(mi * MMCH, (mi + 1) * MMCH)
                    mu_ps = cps.tile([128, MMCH], F32, name="mu_ps",
                                     tag="c_p1")
                    nc.tensor.matmul(mu_ps[:], mean_l0[:], y_bf0[:, ms],
                                     start=True, stop=False)
                    nc.tensor.matmul(mu_ps[:], mean_l1[:], y_bf1[:, ms],
                                     start=False, stop=True)
                    sq_ps = cps.tile([128, MMCH], F32, name="sq_ps",
                                     tag="c_p2")
                    nc.tensor.matmul(sq_ps[:], mean_l0[:], y2_bf0[:, ms],
                                     start=True, stop=False)
                    nc.tensor.matmul(sq_ps[:], mean_l1[:], y2_bf1[:, ms],
                                     start=False, stop=True)
                    # var = E[y^2] - mu^2
                    var_t = cpl.tile([128, MMCH], F32, name="var_t",
                                     tag="var_t")
                    nc.scalar.activation(var_t[:], mu_ps[:], AF.Square)
                    nc.vector.tensor_tensor(var_t[:], sq_ps[:], var_t[:], SUB)
                    nc.scalar.activation(var_t[:], var_t[:], AF.Ln,
                                         bias=eps_col[:])
                    rstd = var_t
                    nc.scalar.activation(rstd[:], var_t[:], AF.Exp, bias=0.0,
                                         scale=-0.5)
                    ymu = cpl.tile([128, MMCH], BF16, name="ymu", tag="ymu")
                    nc.vector.tensor_tensor(ymu[:D0, :], y_bf0[:, ms],
                                            mu_ps[:D0, :], SUB)
                    nc.vector.tensor_tensor(ymu[:D0, :], ymu[:D0, :],
                                            rstd[:D0, :], MUL)
                    nc.vector.tensor_scalar(lny0[:, ms], ymu[:D0, :],
                                            w_lng0[:], w_lnb0[:], MUL, ADD)
                    ymu1 = cpl.tile([D1, MMCH], BF16, name="ymu1", tag="ymu1")
                    nc.vector.tensor_tensor(ymu1[:, :], y_bf1[:, ms],
                                            mu_ps[:D1, :], SUB)
                    nc.gpsimd.tensor_tensor(ymu1[:, :], ymu1[:, :],
                                            rstd[:D1, :], MUL)
                    nc.gpsimd.tensor_scalar(lny1[:, ms], ymu1[:, :],
                                            w_lng1[:], w_lnb1[:], MUL, ADD)
                for oi in range(LCH // 128):
                    ls128 = slice(oi * 128, (oi + 1) * 128)
                    gsl = slice(lc * LCH + oi * 128,
                                lc * LCH + (oi + 1) * 128)
                    pso = cps.tile([128, MMCH], F32, name="pso", tag="c_p2")
                    nc.tensor.matmul(pso[:, 0:C], lny0[:, ls128], w_woutT0[:],
                                     start=True, stop=False)
                    nc.tensor.matmul(pso[:, 0:C], lny1[:, ls128], w_woutT1[:],
                                     start=False, stop=True)
                    res = opool.tile([128, C], F32, name="res", tag="res")
                    nc.sync.dma_start(res[:], xnat_o[gsl, :])
                    outt = opool.tile([128, C], F32, name="outt", tag="outt")
                    nc.vector.tensor_tensor(outt[:], pso[:, 0:C], res[:], ADD)
                    nc.sync.dma_start(out_o[gsl, :], outt[:])

            for lc in range(NCH):
                stage_c_chunk(lc)

    nc.finalize()
    return nc


_CACHE = {}


def _kperm(a):
    return np.ascontiguousarray(
        np.transpose(a, (1, 0, 2)).reshape(a.shape[1], -1)).astype(BF)


def _prep_core_inputs(inputs, b, mod):
    x_own = inputs["x_rgb"] if mod == 0 else inputs["x_e"]
    x_oth = inputs["x_e"] if mod == 0 else inputs["x_rgb"]
    ipw_own = inputs["in_proj_x_w"] if mod == 0 else inputs["in_proj_e_w"]
    ipw_oth = inputs["in_proj_e_w"] if mod == 0 else inputs["in_proj_x_w"]
    cw_own = inputs["conv_x_w"] if mod == 0 else inputs["conv_e_w"]
    cw_oth = inputs["conv_e_w"] if mod == 0 else inputs["conv_x_w"]
    cb_own = inputs["conv_x_b"] if mod == 0 else inputs["conv_e_b"]
    cb_oth = inputs["conv_e_b"] if mod == 0 else inputs["conv_x_b"]
    lng = inputs["ln_r_g"] if mod == 0 else inputs["ln_e_g"]
    lnb = inputs["ln_r_b"] if mod == 0 else inputs["ln_e_b"]
    wout = inputs["out_proj_x_w"] if mod == 0 else inputs["out_proj_e_w"]

    def padT(x):
        xp = np.zeros((C, H + 2, W + 2), np.float32)
        xp[:, 1:H + 1, 1:W + 1] = np.transpose(x, (2, 0, 1))
        return xp.reshape(C, -1).astype(BF)

    def fused_w(ipw, cw):
        wf = np.zeros((9, C, 256), np.float32)
        for tap in range(9):
            dy, dx = tap // 3, tap % 3
            full = ipw.T * cw[:, 0, dy, dx][None, :]
            wf[tap, :, :128] = full[:, :128]
            wf[tap, :, 128:192] = full[:, 128:]
            wf[tap, :, 192:256] = full[:, 128:]
        return np.ascontiguousarray(
            np.transpose(wf, (1, 0, 2)).reshape(C, 9 * 256)).astype(BF)

    def dup256(v):
        out = np.zeros(256, np.float32)
        out[:128] = v[:128]
        out[128:192] = v[128:]
        out[192:256] = v[128:]
        return out

    xpw = inputs["x_proj_weight"]
    dtw = inputs["dt_projs_weight"]
    dtb = inputs["dt_projs_bias"]
    Ds = inputs["Ds"]

    xpw_dt = np.transpose(xpw[:, :R, :], (0, 2, 1))
    xpw_Bp = np.zeros((K, DIN, 128), np.float32)
    xpw_Cp = np.zeros((K, DIN, 128), np.float32)
    for n in range(N):
        xpw_Bp[:, :, 32 * n] = xpw[:, R + n, :]
        xpw_Cp[:, :, 32 * n] = xpw[:, R + N + n, :]
    dtw_t = np.transpose(dtw, (0, 2, 1))
    dtw_d1p = np.concatenate([dtw_t[:, :, 128:], dtw_t[:, :, 128:]], axis=2)
    dtb_d1p = np.concatenate([dtb[:, 128:], dtb[:, 128:]], axis=1)
    dsum = Ds.reshape(K, DIN).sum(axis=0)

    nsa = np.concatenate([np.full(64, -1.0), np.full(64, -2.0)])
    nsb = np.concatenate([np.full(64, -3.0), np.full(64, -4.0)])
    sh = np.zeros((128, 64), np.float32)
    for j in range(64):
        sh[j, j] = 1.0
        sh[64 + j, j] = 1.0

    f32 = np.float32
    cbd_own = dup256(cb_own)
    cbd_oth = dup256(cb_oth)
    return {
        "xpad_o": padT(x_own[b]),
        "xpad_t": padT(x_oth[b]),
        "xnat_o": np.ascontiguousarray(x_own[b].reshape(L, C)).astype(f32),
        "wf_o": fused_w(ipw_own, cw_own),
        "wf_t": fused_w(ipw_oth, cw_oth),
        "cb_o": cbd_own[None, :].astype(BF),
        "cb_t": cbd_oth[None, :].astype(BF),
        "cbc_o": np.stack([cbd_own[:128], cbd_own[128:]], axis=1).astype(f32),
        "cbc_t": np.stack([cbd_oth[:128], cbd_oth[128:]], axis=1).astype(f32),
        "xpw_dt0": _kperm(xpw_dt[:, :128, :]),
        "xpw_dt1": _kperm(xpw_dt[:, 128:, :]),
        "xpw_B0": _kperm(xpw_Bp[:, :128, :]),
        "xpw_B1": _kperm(xpw_Bp[:, 128:, :]),
        "xpw_C0": _kperm(xpw_Cp[:, :128, :]),
        "xpw_C1": _kperm(xpw_Cp[:, 128:, :]),
        "dtw_d0": _kperm(dtw_t[:, :, :128]),
        "dtw_d1p": _kperm(dtw_d1p),
        "dtb_d0": np.ascontiguousarray(dtb[:, :128].T).astype(f32),
        "dtb_d1p": np.ascontiguousarray(dtb_d1p.T).astype(f32),
        "nscale_d1a": nsa[:, None].astype(f32),
        "nscale_d1b": nsb[:, None].astype(f32),
        "ident128": np.eye(128, dtype=f32).astype(BF),
        "shsum": sh.astype(BF),
        "dsum_d0": dsum[:128, None].astype(f32),
        "dsum_d1": dsum[128:, None].astype(f32),
        "ln_g0": lng[:128, None].astype(f32),
        "ln_g1": lng[128:, None].astype(f32),
        "ln_b0": lnb[:128, None].astype(f32),
        "ln_b1": lnb[128:, None].astype(f32),
        "woutT0": np.ascontiguousarray(wout.T[:128, :]).astype(BF),
        "woutT1": np.ascontiguousarray(wout.T[128:, :]).astype(BF),
    }


def kernel(**inputs):
    if "nc" not in _CACHE:
        _CACHE["nc"] = build_nc()
    nc = _CACHE["nc"]
    in_maps = [_prep_core_inputs(inputs, core // 2, core % 2)
               for core in range(NCORE)]
    res = run_bass_kernel_spmd(nc, in_maps, core_ids=list(range(NCORE)))
    _CACHE["last_res"] = res
    out = np.empty((2, B, H, W, C), np.float32)
    for core in range(NCORE):
        b, mod = core // 2, core % 2
        out[mod, b] = res.results[core]["out_o"].reshape(H, W, C)
    return out


if __name__ == "__main__":
    build_nc()
    print("build ok")
